# revision 53
# baseline (speedup 1.0000x reference)
"""Gaussian Splatting 2D on 8 Trainium2 NeuronCores.

Strategy: shard pixels (rows of the image) across the 8 cores; every core
handles all N=1000 gaussians for its 32-row band (embarrassingly parallel, no
collectives).  Inside each core, gaussians that cannot touch the band
(|my - band_center| > 16.5 + 4.7*max(sx,sy), i.e. alpha < ~1e-5 everywhere in
the band) are culled ON DEVICE with an order-preserving compaction, shrinking
the 8 gaussian chunks to C_CULL (typically 4).

Per-core pipeline (gaussians on partitions, pixels on the free dim):
  - u = -q/2 + log(opac) evaluated as ONE fp32 matmul per tile:
      u[g,p] = sum_k coeff[k,g] * basis[k,p],  basis = [x^2, xy, y^2, x, y, 1]
  - la = clamp(u, -87, ln(0.999))      (DVE tensor_scalar, = log alpha)
  - alpha = Exp(la)                    (ACT)
  - lom = Ln(1 - alpha)                (ACT, scale=-1 bias=1)
  - exclusive cumsum over gaussians via matmul with a strict-lower-triangular
    f32r matrix (1 cycle/row); the chunk-to-chunk offset rides in row 126; a
    second accumulating matmul against a diagonal 0/1 matrix folds +log(alpha)
    in, so B = logT + log(alpha) in PSUM.
  - w = Exp(B)                         (ACT)  -> alpha * transmittance
  - img += colors^T @ w                (matmul accumulated in PSUM)

Culling (all on-chip, no DRAM roundtrip): hit in {0,1} per gaussian ->
exclusive prefix sum over the original order (same triangular matmul) ->
destination slot (culled -> out-of-range trash slot) -> selection matrices
P[i, j] = (slot[i] == j) built with one is_equal per source chunk -> tiny
accumulating matmuls gather the 8-value coefficient records into C_CULL
compact chunks.  Order preserved, so compositing is exact; culled gaussians
have alpha < 1e-5 in this band.  Empty tail slots are turned into pad
records (alpha ~ e^-80) by patching cF with the gathered fill indicator.

Exp and Ln share one ACT table set (natural_log_exp_and_others); the
act-table pass is steered so the hot loop performs no table reloads.
"""

import math
import numpy as np

import concourse.bass as bass
import concourse.bacc as bacc
import concourse.hw_specs as hw_specs
import concourse.tile as tile
import concourse.mybir as mybir
from concourse.bass_utils import run_bass_kernel_spmd

F32 = mybir.dt.float32
F32R = mybir.dt.float32r
AF = mybir.ActivationFunctionType
ALU = mybir.AluOpType

W = 256
H = 256
N = 1000
NCORES = 8
ROWS_PER_CORE = H // NCORES          # 32
CHUNK = 126                          # gaussians per chunk (rows 0..125)
NCHUNK = 8                           # 8*126 = 1008 >= 1000
NPAD = CHUNK * NCHUNK                # 1008
TRASH = 4000.0                       # slot for culled gaussians (matches no j)
MACROS = 8                           # 32x32-pixel tiles per core
MACRO_PX = 1024
HALF = 512                           # matmul free-dim (one PSUM bank)
CORE_PX = MACROS * MACRO_PX          # 8192
RECL = 12                            # A,B,C,D,E,F,color,fill,mx,rmax,-,-

CULL_K = 4.7                         # bounding radius in sigmas
CULL_MARGIN = 16.5                   # half band height + 0.5
LN_ALPHA_MAX = float(np.log(np.float32(0.999)))
LA_MIN = -87.0
PAD_OPAC = -80.0
PAD_MY = -1.0e4                      # guarantees pad rows are culled


def _r32(x):
    return np.ascontiguousarray(x, dtype=np.float32)


def _host_blob_r():
    """f32r constants: [tmat | dmat | onescol | onesrow-in-row-0]."""
    tmat = np.zeros((128, 128), dtype=np.float32)
    for m in range(CHUNK):
        tmat[:m, m] = 1.0
        tmat[126, m] = 1.0
    tmat[:CHUNK, 126] = 1.0
    tmat[126, 126] = 1.0
    dmat = np.zeros((128, 128), dtype=np.float32)
    for g in range(CHUNK):
        dmat[g, g] = 1.0
    onescol = np.ones((128, 1), dtype=np.float32)
    onesrow = np.zeros((128, 128), dtype=np.float32)
    onesrow[0, :] = 1.0
    return np.concatenate([tmat, dmat, onescol, onesrow], axis=1)


def _host_pack_params(means, quats, scales, rgbs, opacities):
    """[128, 56]: 7 param planes of [128, 8]; row i, col t = gaussian t*126+i.
    Rows 126..127 + slots >= N are benign padding (culled / alpha ~ e^-80)."""
    def pack(v, pad):
        full = np.full(NPAD, pad, dtype=np.float32)
        full[:N] = v
        out = np.full((128, NCHUNK), pad, dtype=np.float32)
        out[:CHUNK, :] = full.reshape(NCHUNK, CHUNK).T
        return out
    planes = [
        pack(means[:, 0], 0.0),        # mx
        pack(means[:, 1], PAD_MY),     # my
        pack(scales[:, 0], 1.0),       # sx
        pack(scales[:, 1], 1.0),       # sy
        pack(quats, 0.0),              # th
        pack(opacities, PAD_OPAC),     # op
        pack(rgbs[:, 0], 0.0),         # rgb
    ]
    return np.concatenate(planes, axis=1)


def _host_basis(core):
    """[6, 8192] pixel polynomial basis for this core's 32-row band.
    Free-dim order: m*1024 + y_loc*32 + x_loc for macro-tile m (32x32 px)."""
    y0 = core * ROWS_PER_CORE
    xs = np.arange(W, dtype=np.float32) + 0.5
    ys = np.arange(y0, y0 + ROWS_PER_CORE, dtype=np.float32) + 0.5
    basis = np.empty((6, CORE_PX), dtype=np.float32)
    for m in range(MACROS):
        x = xs[m * 32:(m + 1) * 32][None, :]
        y = ys[:, None]
        xg = np.broadcast_to(x, (32, 32)).ravel()
        yg = np.broadcast_to(y, (32, 32)).ravel()
        sl = slice(m * MACRO_PX, (m + 1) * MACRO_PX)
        basis[0, sl] = xg * xg
        basis[1, sl] = xg * yg
        basis[2, sl] = yg * yg
        basis[3, sl] = xg
        basis[4, sl] = yg
        basis[5, sl] = 1.0
    return basis


def _build_module(c_cull, c2):
    nc = bacc.Bacc("TRN2", target_bir_lowering=False, debug=False)
    cap = c_cull * CHUNK
    blob_f_w = 56 + 1 + 128 + cap + MACROS * c_cull + 2

    ins = {}
    for name, shape, dt in [
        ("basis", (6, CORE_PX), F32),
        ("blob_f", (128, blob_f_w), F32),
        ("blob_r", (128, 385), F32R),
    ]:
        ins[name] = nc.dram_tensor(name, list(shape), dt, kind="ExternalInput")
    out_dram = nc.dram_tensor("img_out", [1, CORE_PX], F32, kind="ExternalOutput")

    with tile.TileContext(nc) as tc:
        _build_kernel(tc, ins, out_dram, c_cull, c2)

    # The activation-table-load pass picks the FIRST set containing each
    # func, which alternates natural_log <-> exp_and_others in the hot loop
    # (one ~1.3us table DMA per activation!).  Restrict its view to three
    # sets -- keeping list positions so act_func_set_id stays aligned with
    # act_info.json -- so exp+ln land together in natural_log_exp_and_others.
    keep = {"trig_and_small", "sigmoid_and_others", "natural_log_exp_and_others"}
    real = hw_specs.get_activation_tables(nc.m.arch)
    filtered = {name: (funcs if name in keep else frozenset())
                for name, funcs in real.items()}
    orig = bacc.get_activation_tables
    bacc.get_activation_tables = lambda arch: filtered
    try:
        nc.compile()
    finally:
        bacc.get_activation_tables = orig
    return nc


def _build_kernel(tc, ins, out_dram, c_cull, c2):
    nc = tc.nc
    cap = c_cull * CHUNK

    with (
        tc.tile_pool(name="consts", bufs=1) as consts,
        tc.tile_pool(name="setup", bufs=1) as setup,
    ):
        blob_f = consts.tile([128, 56 + 1 + 128 + cap + MACROS * c_cull + 2],
                             F32, tag="blob_f", name="blob_f")
        nc.sync.dma_start(blob_f[:, 0:59], ins["blob_f"][:, 0:59])
        nc.sync.dma_start(blob_f[:, 59:], ins["blob_f"][:, 59:])
        blob_r = consts.tile([128, 385], F32R, tag="blob_r", name="blob_r")
        nc.gpsimd.dma_start(blob_r[:], ins["blob_r"][:])
        basis = consts.tile([6, CORE_PX], F32, tag="basis", name="basis")
        nc.gpsimd.dma_start(basis[:], ins["basis"][:])

        Pn = ("mx", "my", "sx", "sy", "th", "op", "rgb")
        P = {k: blob_f[:, i * NCHUNK:(i + 1) * NCHUNK] for i, k in enumerate(Pn)}
        bandc = blob_f[:, 56:57]
        bias_negpi = blob_f[:, 57:58]
        bias_nhpi = blob_f[:, 58:59]
        ident = blob_f[:, 59:59 + 128]
        iota = blob_f[:, 187:187 + cap]
        cxb = blob_f[:, 187 + cap:187 + cap + MACROS * c_cull]
        tmat = blob_r[:, 0:128]
        dmat = blob_r[:, 128:256]
        onescol = blob_r[:, 256:257]
        onesrow = blob_r[0:1, 257:385]

        # ---- per-gaussian coefficient math on [128, 8] tiles ----
        ntmp = [0]

        def tmp(dt=F32):
            ntmp[0] += 1
            return setup.tile([128, NCHUNK], dt, tag=f"tmp{ntmp[0]}",
                              name=f"tmp{ntmp[0]}")

        def act(func, in_, bias=0.0, scale=1.0, dt=F32):
            o = tmp(dt)
            nc.scalar.activation(o[:], in_[:], func, bias=bias, scale=scale)
            return o

        def vmul(a, b):
            o = tmp()
            nc.vector.tensor_mul(o[:], a[:], b[:])
            return o

        def vadd(a, b):
            o = tmp()
            nc.vector.tensor_add(o[:], a[:], b[:])
            return o

        def vsub(a, b):
            o = tmp()
            nc.vector.tensor_sub(o[:], a[:], b[:])
            return o

        def vsmul(a, s):
            o = tmp()
            nc.vector.tensor_scalar_mul(o[:], a[:], s)
            return o

        # Sin table domain is [-pi, pi] and theta is in [0, 2pi), so use the
        # exact identities sin(t) = -Sin(t - pi), cos(t) = Sin(|t-pi| - pi/2).
        # sq is -sin(theta); the sign cancels in s2 and is absorbed in a12 by
        # swapping the (sx2 - sy2) operands.
        sq = act(AF.Sin, P["th"], bias=bias_negpi)
        zabs = act(AF.Abs, P["th"], bias=bias_negpi)
        cq = act(AF.Sin, zabs, bias=bias_nhpi)
        # force the sigmoid/ln ops to schedule after the trig block so the
        # ACT table set loads exactly once per set (zero-valued dep on cq)
        zz = vsmul(cq, 0.0)
        opg = vadd(P["op"], zz)
        rgg = vadd(P["rgb"], zz)
        sg = act(AF.Sigmoid, opg)
        colors = act(AF.Sigmoid, rgg)
        lo = act(AF.Ln, sg)

        sx2 = vmul(P["sx"], P["sx"])
        sy2 = vmul(P["sy"], P["sy"])
        cq2 = vmul(cq, cq)
        sq2 = vmul(sq, sq)
        cs = vmul(cq, sq)
        a11 = vadd(vmul(cq2, sx2), vmul(sq2, sy2))
        a12 = vmul(cs, vsub(sy2, sx2))
        a22 = vadd(vmul(sq2, sx2), vmul(cq2, sy2))
        det = vsub(vmul(a11, a22), vmul(a12, a12))
        idet = tmp()
        nc.vector.reciprocal(idet[:], det[:])
        nhidet = vsmul(idet, -0.5)
        cA = vmul(a22, nhidet)                       # -ia/2
        cC = vmul(a11, nhidet)                       # -ic/2
        cB = vmul(a12, idet)                         # -ib  (xy coefficient)
        cD = vsub(vsmul(vmul(cA, P["mx"]), -2.0), vmul(cB, P["my"]))
        cE = vsub(vsmul(vmul(cC, P["my"]), -2.0), vmul(cB, P["mx"]))
        mx2 = vmul(P["mx"], P["mx"])
        my2 = vmul(P["my"], P["my"])
        mxy = vmul(P["mx"], P["my"])
        cF = vadd(vadd(vmul(cA, mx2), vmul(cB, mxy)),
                  vadd(vmul(cC, my2), lo))

        # ---- visibility: hit = 1 iff the gaussian can touch this row band --
        rmax = tmp()
        nc.vector.tensor_tensor(rmax[:], P["sx"][:], P["sy"][:], ALU.max)
        dy0 = tmp()
        nc.vector.tensor_scalar(dy0[:], P["my"][:], bandc, None, ALU.subtract)
        dya = act(AF.Abs, dy0)
        kr = vsmul(rmax, CULL_K)
        t5 = tmp()
        nc.vector.tensor_tensor(t5[:], kr[:], dya[:], ALU.subtract)
        hit = tmp(F32R)
        nc.vector.tensor_scalar(hit[:], t5[:], -CULL_MARGIN, None, ALU.is_ge)

        # ---- records: [A,B,C,D,E,F,color,1,mx,rmax,-,-] per gaussian ----
        crec = consts.tile([128, NCHUNK * RECL], F32, tag="crec", name="crec")
        nc.vector.memset(crec[:], 1.0)
        for ci, csrc in enumerate((cA, cB, cC, cD, cE, cF, colors)):
            nc.vector.tensor_copy(crec[:, ci:NCHUNK * RECL:RECL], csrc[:])
        nc.vector.tensor_copy(crec[:, 8:NCHUNK * RECL:RECL], P["mx"])
        nc.vector.tensor_copy(crec[:, 9:NCHUNK * RECL:RECL], rmax[:])

        # ---- order-preserving slots: exclusive prefix sum of hit ----
        with tc.tile_pool(name="psum_setup", bufs=2,
                          space=bass.MemorySpace.PSUM) as psetup:
            pos_ps = psetup.tile([128, NCHUNK], F32, tag="pos", name="pos", bufs=1)
            nc.tensor.matmul(pos_ps[:], tmat[0:CHUNK, :], hit[0:CHUNK, :],
                             start=True, stop=False)
            tot_ps = psetup.tile([1, NCHUNK], F32, tag="tot", name="tot", bufs=1)
            nc.tensor.matmul(tot_ps[:], onescol[0:CHUNK, :], hit[0:CHUNK, :],
                             start=True, stop=True)
            tot = setup.tile([1, NCHUNK], F32, tag="tot_sb", name="tot_sb")
            nc.vector.tensor_copy(tot[:], tot_ps[:])
            coloff = setup.tile([1, NCHUNK], F32R, tag="coloff", name="coloff")
            nc.vector.tensor_scalar(coloff[0:1, 0:1], tot[0:1, 0:1], 0.0,
                                    None, ALU.mult)
            for t in range(1, NCHUNK):
                nc.vector.tensor_add(coloff[0:1, t:t + 1],
                                     coloff[0:1, t - 1:t], tot[0:1, t - 1:t])
            nc.tensor.matmul(pos_ps[:], onesrow, coloff[:],
                             start=False, stop=True)

            # slot = hit ? pos : TRASH   (TRASH matches no destination)
            p1 = tmp()
            nc.vector.tensor_scalar(p1[:], pos_ps[:], TRASH, None, ALU.subtract)
            p2 = tmp()
            nc.vector.tensor_mul(p2[:], p1[:], hit[:])
            slotf = tmp()
            nc.vector.tensor_scalar(slotf[:], p2[:], TRASH, None, ALU.add)

            # ---- level 1: selection-matmul compaction into c_cull chunks ---
            pmats = []
            for t in range(NCHUNK):
                pmat = setup.tile([128, cap], F32, tag="pm", name="pm",
                                  bufs=NCHUNK)
                nc.vector.tensor_scalar(pmat[:], iota[:, 0:cap],
                                         slotf[:, t:t + 1], None, ALU.is_equal)
                pmats.append(pmat)
            cos = []
            for c in range(c_cull):
                cc = psetup.tile([CHUNK, RECL], F32, tag="cc", name="cc",
                                 bufs=1)
                for t in range(NCHUNK):
                    nc.tensor.matmul(cc[:],
                                     pmats[t][:, c * CHUNK:(c + 1) * CHUNK],
                                     crec[:, t * RECL:(t + 1) * RECL],
                                     start=(t == 0), stop=(t == NCHUNK - 1))
                co = consts.tile([CHUNK, RECL], F32, tag=f"co{c}",
                                 name=f"co{c}")
                nc.vector.tensor_copy(co[:], cc[:])
                # empty tail slots: fill=0 -> force cF to PAD_OPAC and push
                # mx far away so level 2 can never pick the slot up
                fix = setup.tile([CHUNK, 1], F32, tag=f"fx{c}", name=f"fx{c}")
                nc.vector.tensor_scalar(fix[:], co[:, 7:8], -PAD_OPAC,
                                        PAD_OPAC, ALU.mult, ALU.add)
                nc.vector.tensor_add(co[:, 5:6], co[:, 5:6], fix[:])
                fxm = setup.tile([CHUNK, 1], F32, tag=f"fm{c}", name=f"fm{c}")
                nc.vector.tensor_scalar(fxm[:], co[:, 7:8], -PAD_MY,
                                        PAD_MY, ALU.mult, ALU.add)
                nc.vector.tensor_add(co[:, 8:9], co[:, 8:9], fxm[:])
                cos.append(co)

            # ---- level 2: per-macro-column compaction into c2 chunks ----
            # mx2/kr2: [126, c_cull] views of the level-1 compacted set
            mx2 = setup.tile([CHUNK, c_cull], F32, tag="mx2", name="mx2")
            kr2 = setup.tile([CHUNK, c_cull], F32, tag="kr2", name="kr2")
            for c in range(c_cull):
                nc.vector.tensor_copy(mx2[:, c:c + 1], cos[c][:, 8:9])
                nc.vector.tensor_scalar(kr2[:, c:c + 1], cos[c][:, 9:10],
                                        CULL_K, None, ALU.mult)

            cap2 = c2 * CHUNK
            ncol = MACROS * c_cull
            # batched level-2 hit test / prefix scan for ALL 8 columns at
            # once; free index = m*c_cull + c
            mxr = setup.tile([CHUNK, ncol], F32, tag="mxr", name="mxr")
            krr = setup.tile([CHUNK, ncol], F32, tag="krr", name="krr")
            for m in range(MACROS):
                nc.vector.tensor_copy(mxr[:, m * c_cull:(m + 1) * c_cull],
                                      mx2[:])
                nc.vector.tensor_copy(krr[:, m * c_cull:(m + 1) * c_cull],
                                      kr2[:])
            dx0a = setup.tile([CHUNK, ncol], F32, tag="dx0a", name="dx0a")
            nc.vector.tensor_tensor(dx0a[:], mxr[:], cxb[0:CHUNK, :],
                                    ALU.subtract)
            dxaa = setup.tile([CHUNK, ncol], F32, tag="dxaa", name="dxaa")
            nc.scalar.activation(dxaa[:], dx0a[:], AF.Abs)
            txa = setup.tile([CHUNK, ncol], F32, tag="txa", name="txa")
            nc.vector.tensor_tensor(txa[:], krr[:], dxaa[:], ALU.subtract)
            hit2a = setup.tile([CHUNK, ncol], F32R, tag="hit2a", name="hit2a")
            nc.vector.tensor_scalar(hit2a[:], txa[:], -CULL_MARGIN, None,
                                    ALU.is_ge)

            pos2a = psetup.tile([128, ncol], F32, tag="pos2", name="pos2",
                                bufs=1)
            nc.tensor.matmul(pos2a[:], tmat[0:CHUNK, :], hit2a[:],
                             start=True, stop=False)
            tot2p = psetup.tile([1, ncol], F32, tag="tot2", name="tot2",
                                bufs=1)
            nc.tensor.matmul(tot2p[:], onescol[0:CHUNK, :], hit2a[:],
                             start=True, stop=True)
            tot2 = setup.tile([1, ncol], F32, tag="tot2s", name="tot2s")
            nc.vector.tensor_copy(tot2[:], tot2p[:])
            coff2 = setup.tile([1, ncol], F32R, tag="coff2", name="coff2")
            nc.vector.tensor_scalar(coff2[0:1, 0:ncol:c_cull],
                                    tot2[0:1, 0:ncol:c_cull], 0.0,
                                    None, ALU.mult)
            for c in range(1, c_cull):
                nc.vector.tensor_add(coff2[0:1, c:ncol:c_cull],
                                     coff2[0:1, c - 1:ncol:c_cull],
                                     tot2[0:1, c - 1:ncol:c_cull])
            nc.tensor.matmul(pos2a[:], onesrow, coff2[:],
                             start=False, stop=True)

            q1a = setup.tile([CHUNK, ncol], F32, tag="q1a", name="q1a")
            nc.vector.tensor_scalar(q1a[:], pos2a[0:CHUNK, :], TRASH, None,
                                    ALU.subtract)
            q2a = setup.tile([CHUNK, ncol], F32, tag="q2a", name="q2a")
            nc.vector.tensor_mul(q2a[:], q1a[:], hit2a[:])
            sl2a = setup.tile([CHUNK, ncol], F32, tag="sl2a", name="sl2a")
            nc.vector.tensor_scalar(sl2a[:], q2a[:], TRASH, None, ALU.add)

            qcT2 = [[None] * c2 for _ in range(MACROS)]
            colT2 = [[None] * c2 for _ in range(MACROS)]
            for m in range(MACROS):
                pm2s = []
                for c in range(c_cull):
                    pm2 = setup.tile([CHUNK, cap2], F32, tag="pm2",
                                     name="pm2", bufs=3 * c_cull)
                    eng = nc.gpsimd if (c % 2) else nc.vector
                    eng.tensor_scalar(
                        pm2[:], iota[0:CHUNK, 0:cap2],
                        sl2a[:, m * c_cull + c:m * c_cull + c + 1], None,
                        ALU.is_equal)
                    pm2s.append(pm2)
                for c2i in range(c2):
                    cc2 = psetup.tile([CHUNK, RECL], F32, tag="cc2",
                                      name="cc2", bufs=2)
                    for c in range(c_cull):
                        nc.tensor.matmul(
                            cc2[:],
                            pm2s[c][:, c2i * CHUNK:(c2i + 1) * CHUNK],
                            cos[c][:],
                            start=(c == 0), stop=(c == c_cull - 1))
                    co2 = consts.tile([CHUNK, RECL], F32,
                                      tag=f"co2_{m}_{c2i}",
                                      name=f"co2_{m}_{c2i}")
                    nc.vector.tensor_copy(co2[:], cc2[:])
                    fix2 = setup.tile([CHUNK, 1], F32, tag="fix2",
                                      name="fix2", bufs=2)
                    nc.vector.tensor_scalar(fix2[:], co2[:, 7:8], -PAD_OPAC,
                                            PAD_OPAC, ALU.mult, ALU.add)
                    nc.vector.tensor_add(co2[:, 5:6], co2[:, 5:6], fix2[:])

                    tr = psetup.tile([6, CHUNK], F32, tag="tr", name="tr",
                                     bufs=1)
                    nc.tensor.transpose(tr[:], co2[0:CHUNK, 0:6],
                                        ident[0:CHUNK, 0:CHUNK])
                    q = consts.tile([6, CHUNK], F32, tag=f"q2_{m}_{c2i}",
                                    name=f"q2_{m}_{c2i}")
                    nc.vector.tensor_copy(q[:], tr[:])
                    qcT2[m][c2i] = q
                    col = consts.tile([CHUNK, 1], F32R, tag=f"cl2_{m}_{c2i}",
                                      name=f"cl2_{m}_{c2i}")
                    nc.vector.tensor_copy(col[:], co2[0:CHUNK, 6:7])
                    colT2[m][c2i] = col

        # ---- main loop over compacted chunks ----
        with (
            tc.tile_pool(name="psumA", bufs=1, space=bass.MemorySpace.PSUM) as pA,
            tc.tile_pool(name="psumB", bufs=2, space=bass.MemorySpace.PSUM) as pB,
            tc.tile_pool(name="psumC", bufs=1, space=bass.MemorySpace.PSUM) as pC,
            tc.tile_pool(name="work", bufs=3) as work,
        ):
            img_all = work.tile([1, CORE_PX], F32, tag="imga", name="imga",
                                bufs=1)
            for m in range(MACROS):
                cimg = pC.tile([1, MACRO_PX], F32, tag="C", name="C")
                bprev = None
                for t in range(c2):
                    ua = pA.tile([CHUNK, MACRO_PX], F32, tag="A", name="A")
                    for s in range(2):
                        nc.tensor.matmul(
                            ua[:, s * HALF:(s + 1) * HALF],
                            qcT2[m][t][:],
                            basis[0:6, m * MACRO_PX + s * HALF:
                                  m * MACRO_PX + (s + 1) * HALF],
                            start=True, stop=True)

                    la = work.tile([CHUNK, MACRO_PX], F32R, tag="la", name="la",
                                   bufs=4)
                    nc.vector.tensor_scalar(
                        la[:], ua[:], LN_ALPHA_MAX, LA_MIN, ALU.min, ALU.max)

                    alpha = work.tile([CHUNK, MACRO_PX], F32, tag="alpha",
                                      name="alpha", bufs=4)
                    nc.scalar.activation(alpha[:], la[:], AF.Exp)

                    staged = work.tile([128, MACRO_PX], F32R, tag="staged",
                                       name="staged", bufs=4)
                    if t > 0:
                        # offset row lives at partition 126; engine APs must
                        # start at 0/32/64/96, so copy partitions 96..126 and
                        # let the Ln overwrite rows 96..125 with real data.
                        nc.scalar.copy(staged[96:127, :], bprev[96:127, :])
                    nc.scalar.activation(staged[0:CHUNK, :], alpha[:],
                                         AF.Ln, bias=1.0, scale=-1.0)

                    bt = pB.tile([128, MACRO_PX], F32, tag="B", name="B")
                    kk = CHUNK if t == 0 else CHUNK + 1   # row 126 = offset
                    for s in range(2):
                        sl = slice(s * HALF, (s + 1) * HALF)
                        nc.tensor.matmul(bt[:, sl], tmat[0:kk, :],
                                         staged[0:kk, sl],
                                         start=True, stop=False)
                    for s in range(2):
                        sl = slice(s * HALF, (s + 1) * HALF)
                        nc.tensor.matmul(bt[:, sl], dmat[0:CHUNK, :],
                                         la[:, sl],
                                         start=False, stop=True)

                    wt = work.tile([CHUNK, MACRO_PX], F32R, tag="w", name="w",
                                   bufs=4)
                    nc.scalar.activation(wt[:], bt[0:CHUNK, :], AF.Exp)

                    for s in range(2):
                        sl = slice(s * HALF, (s + 1) * HALF)
                        nc.tensor.matmul(cimg[0:1, sl], colT2[m][t][:],
                                         wt[:, sl],
                                         start=(t == 0), stop=(t == c2 - 1))
                    bprev = bt

                nc.vector.tensor_copy(
                    img_all[0:1, m * MACRO_PX:(m + 1) * MACRO_PX], cimg[:])
            nc.sync.dma_start(out_dram[:], img_all[:])


_MODULE_CACHE = {}


def _get_module(c_cull, c2):
    key = (c_cull, c2)
    if key not in _MODULE_CACHE:
        _MODULE_CACHE[key] = _build_module(c_cull, c2)
    return _MODULE_CACHE[key]


def _pick_c_cull(means, scales):
    """Capacities for the per-core (level 1) and per-column (level 2) visible
    sets; the host hit tests are 1px looser than the device ones, so host
    count >= device count under fp rounding."""
    mx, my = means[:, 0], means[:, 1]
    r = CULL_K * np.maximum(scales[:, 0], scales[:, 1])
    vmax = 0
    v2max = 0
    for core in range(NCORES):
        cy = core * ROWS_PER_CORE + ROWS_PER_CORE / 2.0
        band = np.abs(my - cy) <= (CULL_MARGIN + 1.0) + r
        vmax = max(vmax, int(band.sum()))
        for m in range(MACROS):
            cx = m * 32 + 16.0
            colhit = band & (np.abs(mx - cx) <= (CULL_MARGIN + 1.0) + r)
            v2max = max(v2max, int(colhit.sum()))
    c_cull = min(NCHUNK, max(1, -(-(vmax + 4) // CHUNK)))
    c2 = min(c_cull, max(1, -(-(v2max + 4) // CHUNK)))
    return c_cull, c2


def _make_in_maps(c_cull, means, quats, scales, rgbs, opacities):
    cap = c_cull * CHUNK
    par7 = _host_pack_params(means, quats, scales, rgbs, opacities)
    blob_r = _host_blob_r()
    ident = np.eye(128, dtype=np.float32)
    iota = np.broadcast_to(np.arange(cap, dtype=np.float32)[None, :],
                           (128, cap)).copy()
    centers = np.repeat(np.arange(MACROS, dtype=np.float32) * 32 + 16, c_cull)
    cxb = np.broadcast_to(centers[None, :], (128, MACROS * c_cull)).copy()
    in_maps = []
    for core in range(NCORES):
        cy = np.full((128, 1), core * ROWS_PER_CORE + ROWS_PER_CORE / 2.0,
                     dtype=np.float32)
        b1 = np.full((128, 1), -np.pi, dtype=np.float32)
        b2 = np.full((128, 1), -np.pi / 2.0, dtype=np.float32)
        blob_f = np.concatenate([par7, cy, b1, b2, ident, iota, cxb], axis=1)
        in_maps.append({"basis": _host_basis(core), "blob_f": blob_f,
                        "blob_r": blob_r})
    return in_maps


def _assemble(results):
    img = np.empty((H, W), dtype=np.float32)
    for core in range(NCORES):
        flat = np.asarray(results[core]["img_out"]).reshape(CORE_PX)
        band = flat.reshape(MACROS, 32, 32).transpose(1, 0, 2).reshape(32, W)
        img[core * ROWS_PER_CORE:(core + 1) * ROWS_PER_CORE, :] = band
    return img[None, None]


def run(means, quats, scales, rgbs, opacities, trace=False):
    means = _r32(means)
    quats = _r32(quats)
    scales = _r32(scales)
    rgbs = _r32(np.asarray(rgbs).reshape(np.asarray(rgbs).shape[0], -1))
    opacities = _r32(opacities)
    c_cull, c2 = _pick_c_cull(means, scales)
    nc = _get_module(c_cull, c2)
    in_maps = _make_in_maps(c_cull, means, quats, scales, rgbs, opacities)
    res = run_bass_kernel_spmd(nc, in_maps, list(range(NCORES)), trace=trace)
    return _assemble(res.results), res.exec_time_ns


def kernel(means, quats, scales, rgbs, opacities):
    out, _ = run(means, quats, scales, rgbs, opacities)
    return out
